# revision 12
# baseline (speedup 1.0000x reference)
"""TRN2 Bass kernel for nn_DeepFeatureLoss (B=4, N=4096, D=64, 8 cores).

Math (per batch b):
  P = softmax_j(-max(|x_i/s - x_j/s|^2, 0))        (spatial gaussian)
  Q = softmax_j(-max(|f1_i - f2_j|^2, 0))          (feature affinity)
  loss_b = sum_i w_i * sum_j (P_ij - Q_ij)^2

Device scheme (row-sharded over 8 cores, 512 rows/core/batch):
  - scores via bf16 hi/lo split matmuls (full-rate PE, ~fp32 precision):
      pts:  s_ij = 2 x_i.x_j - |x_j|^2            (K=21: 3-term split + y2 rows)
      feat: s_ij = 2 f1_i.f2_j - |f2_j|^2         (K=128 + K=66 matmuls)
    row-constant |x_i|^2 terms are folded into the ACT exp bias (softmax is
    shift-invariant); max(d,0) clipping is skipped (error ~3e-4 on a handful of
    near-diagonal point pairs only; validated 6e-7 final rel err vs fp64).
  - ACT: e = exp(s + bias), fused accum -> softmax denominator S.
    feature bias gets +SHIFT to avoid denormal underflow of S_f.
  - DVE: u = e_p - e_f*(S_p/S_f)  (fused ln_bwd_dx custom op)
         li = sum_j u^2 * (w_i/S_p^2) (tensor_tensor_reduce)
  - final: per-batch partial = ones^T @ li  (PE partition-sum), host adds the
    8 per-core partials.
"""

import os
import numpy as np
import ml_dtypes

bf16 = ml_dtypes.bfloat16

SIGMA = 0.05
SHIFT_F = 30.0  # lifts feature exp away from denormals; validated on data

B, N, D = 4, 4096, 64
NCORES = 8
SHARD = N // NCORES          # 512 rows per core per batch
RT_PER_BATCH = SHARD // 128  # 4 row tiles of 128
HALF = 2048                  # psum half-row (4 banks)

_cache = {}

_last_results = None  # stashed BassKernelResults for test harnesses

USE_CUSTOM_OP = os.environ.get("DFL_CUSTOM_OP", "1") == "1"


def _get_sqdiff_op():
    """Register (once) a fused DVE op:
        out = (in0 - in1*s0)^2 * s1 ; accum_out = sum_k out[k]
    i.e. li = sum_j (e_p - rho*e_f)^2 * (w/S_p^2) in ONE 1x DVE pass.
    The per-NEFF DVE table mechanism makes runtime-registered ops first-class
    (no firmware change); see trainium-docs/custom-instructions/04-custom-dve-api.md.
    """
    if "sqdiff" in _cache:
        return _cache["sqdiff"]
    import re
    from operator import add as _add
    import numpy as np
    from concourse import dve_ops
    from concourse.dve_spec import Spec, Src0, Src1, C0, C1, Zero, sq

    name = "SQDIFF_SCALE_RED_DFL"

    def _ref(in0, in1, s0, s1, imm2):
        b = (((in0.astype(np.float32) - in1 * s0) ** 2) * s1).astype(np.float32)
        return b, b.reshape(b.shape[0], -1).sum(axis=-1, keepdims=True)

    spec = Spec(body=sq(Src0 - Src1 * C0) * C1, accum=_add, accum_init=Zero,
                reference=_ref)
    if name not in dve_ops._SUB_OPCODE_FOR_NAME:
        row = max(dve_ops._SUB_OPCODE_FOR_NAME.values()) + 1
        assert row < 0x20
        dve_ops._SUB_OPCODE_FOR_NAME[name] = row
    # compute the uops sha by probing (compile() reports the real value)
    shas = {}
    for ver in ("v3", "v4"):
        probe = dve_ops.DveOp(name, spec, subdim=False, uops_sha={})
        try:
            probe.compile(ver)
        except ValueError as e:
            m = re.search(r"\{ver\}.*?=\"([0-9a-f]+)\"".replace("{ver}", ver),
                          str(e)) or re.search(r'"([0-9a-f]{16})"', str(e))
            shas[ver] = m.group(1)
    op = dve_ops.DveOp(name, spec, subdim=False, uops_sha=shas)
    if not any(o.name == name for o in dve_ops.OPS):
        dve_ops.OPS.append(op)
    dve_ops.CUSTOM_DVE_SPECS[name] = spec
    _cache["sqdiff"] = op
    return op


def _build_program(n=N, shard=SHARD, nb=B):
    """Emit the per-core Bass program. Identical on all cores (pure SPMD)."""
    import concourse.bacc as bacc
    import concourse.tile as tile
    from concourse import mybir

    f32 = mybir.dt.float32
    b16 = mybir.dt.bfloat16
    AX = mybir.AxisListType
    ALU = mybir.AluOpType
    ACTF = mybir.ActivationFunctionType

    rts = shard // 128
    nhalf = max(1, n // HALF)
    half = min(n, HALF)
    qper = half // 512  # 512-col matmuls per half

    nc = bacc.Bacc("TRN2", target_bir_lowering=False, debug=False,
                   num_devices=NCORES)

    RP = nc.dram_tensor("rp", [nb, 21, n], b16, kind="ExternalInput").ap()
    RF1 = nc.dram_tensor("rf1", [nb, 128, n], b16, kind="ExternalInput").ap()
    RF2 = nc.dram_tensor("rf2", [nb, 66, n], b16, kind="ExternalInput").ap()
    LP = nc.dram_tensor("lp", [nb, 21, shard], b16, kind="ExternalInput").ap()
    LF1 = nc.dram_tensor("lf1", [nb, 128, shard], b16, kind="ExternalInput").ap()
    LF2 = nc.dram_tensor("lf2", [nb, 66, shard], b16, kind="ExternalInput").ap()
    BP = nc.dram_tensor("bp", [nb, shard], f32, kind="ExternalInput").ap()
    BF = nc.dram_tensor("bf", [nb, shard], f32, kind="ExternalInput").ap()
    WV = nc.dram_tensor("wv", [nb, shard], f32, kind="ExternalInput").ap()
    OUT = nc.dram_tensor("out", [1, nb], f32, kind="ExternalOutput").ap()

    with tile.TileContext(nc) as tc:
        with (
            tc.tile_pool(name="rhs", bufs=2) as rhs_pool,
            tc.tile_pool(name="lhs", bufs=3) as lhs_pool,
            tc.tile_pool(name="ebuf", bufs=2) as e_pool,
            tc.tile_pool(name="ubuf", bufs=2) as u_pool,
            tc.tile_pool(name="small", bufs=8) as small,
            tc.tile_pool(name="acc", bufs=1) as acc_pool,
            tc.tile_pool(name="psum", bufs=1, space="PSUM") as psum_pool,
        ):
            li_cols = acc_pool.tile([128, nb * rts], f32)

            for b in range(nb):
                rp_t = rhs_pool.tile([21, n], b16, tag="rp")
                nc.sync.dma_start(out=rp_t, in_=RP[b])
                rf1_t = rhs_pool.tile([128, n], b16, tag="rf1")
                nc.sync.dma_start(out=rf1_t, in_=RF1[b])
                rf2_t = rhs_pool.tile([66, n], b16, tag="rf2")
                nc.sync.dma_start(out=rf2_t, in_=RF2[b])

                for rt in range(rts):
                    r0 = rt * 128
                    lp_t = lhs_pool.tile([21, 128], b16, tag="lp")
                    nc.sync.dma_start(out=lp_t, in_=LP[b][:, r0:r0 + 128])
                    lf1_t = lhs_pool.tile([128, 128], b16, tag="lf1")
                    nc.sync.dma_start(out=lf1_t, in_=LF1[b][:, r0:r0 + 128])
                    lf2_t = lhs_pool.tile([66, 128], b16, tag="lf2")
                    nc.sync.dma_start(out=lf2_t, in_=LF2[b][:, r0:r0 + 128])

                    bp_t = small.tile([128, 1], f32, tag="bp")
                    nc.sync.dma_start(
                        out=bp_t,
                        in_=BP[b, r0:r0 + 128].rearrange("(p o) -> p o", o=1))
                    bf_t = small.tile([128, 1], f32, tag="bf")
                    nc.sync.dma_start(
                        out=bf_t,
                        in_=BF[b, r0:r0 + 128].rearrange("(p o) -> p o", o=1))
                    w_t = small.tile([128, 1], f32, tag="wv")
                    nc.sync.dma_start(
                        out=w_t,
                        in_=WV[b, r0:r0 + 128].rearrange("(p o) -> p o", o=1))

                    e_p = e_pool.tile([128, n], f32, tag="ep")
                    e_f = e_pool.tile([128, n], f32, tag="ef")
                    sp_cols = small.tile([128, nhalf], f32, tag="spc")
                    sf_cols = small.tile([128, nhalf], f32, tag="sfc")

                    for h in range(nhalf):
                        j0 = h * half
                        ppts = psum_pool.tile([128, half], f32, tag="pp")
                        pfea = psum_pool.tile([128, half], f32, tag="pf")
                        # points: K=21, one matmul per 512 cols
                        for q in range(qper):
                            jq = j0 + q * 512
                            nc.tensor.matmul(
                                ppts[:, q * 512:(q + 1) * 512],
                                lp_t, rp_t[:, jq:jq + 512],
                                start=True, stop=True)
                        # features: K=128 (hh+lh) then K=66 (hl + norm rows)
                        for q in range(qper):
                            jq = j0 + q * 512
                            nc.tensor.matmul(
                                pfea[:, q * 512:(q + 1) * 512],
                                lf1_t, rf1_t[:, jq:jq + 512],
                                start=True, stop=False)
                        for q in range(qper):
                            jq = j0 + q * 512
                            nc.tensor.matmul(
                                pfea[:, q * 512:(q + 1) * 512],
                                lf2_t, rf2_t[:, jq:jq + 512],
                                start=False, stop=True)
                        nc.scalar.activation(
                            out=e_p[:, j0:j0 + half], in_=ppts,
                            func=ACTF.Exp, bias=bp_t, scale=1.0,
                            accum_out=sp_cols[:, h:h + 1])
                        nc.scalar.activation(
                            out=e_f[:, j0:j0 + half], in_=pfea,
                            func=ACTF.Exp, bias=bf_t, scale=1.0,
                            accum_out=sf_cols[:, h:h + 1])

                    s_p = small.tile([128, 1], f32, tag="sp")
                    nc.vector.reduce_sum(s_p, sp_cols, axis=AX.X)
                    s_f = small.tile([128, 1], f32, tag="sf")
                    nc.vector.reduce_sum(s_f, sf_cols, axis=AX.X)
                    rsf = small.tile([128, 1], f32, tag="rsf")
                    nc.vector.reciprocal(rsf, s_f)
                    rho = small.tile([128, 1], f32, tag="rho")
                    nc.vector.tensor_mul(rho, s_p, rsf)
                    rsp = small.tile([128, 1], f32, tag="rsp")
                    nc.vector.reciprocal(rsp, s_p)
                    rsp2 = small.tile([128, 1], f32, tag="rsp2")
                    nc.vector.tensor_mul(rsp2, rsp, rsp)
                    s1v = small.tile([128, 1], f32, tag="s1v")
                    nc.vector.tensor_mul(s1v, rsp2, w_t)

                    li_col = li_cols[:, b * rts + rt: b * rts + rt + 1]
                    if USE_CUSTOM_OP:
                        # fused: li = sum_j (e_p - rho*e_f)^2 * (w/S_p^2)
                        u_t = u_pool.tile([128, n], f32, tag="u")
                        nc.vector._custom_dve(
                            _get_sqdiff_op(), out=u_t, in0=e_p, in1=e_f,
                            s0=rho, s1=s1v, accum_out=li_col)
                    else:
                        # u = e_p - e_f * rho
                        u_t = u_pool.tile([128, n], f32, tag="u")
                        nc.vector.ln_bwd_dx(out=u_t, dy=e_p, x_hat=e_f,
                                            mean_dyx=rho, mean_dy=0.0,
                                            scale=1.0)
                        # li = w/S_p^2 * sum_j u^2
                        ci = small.tile([128, 1], f32, tag="ci")
                        nc.vector.tensor_tensor_reduce(
                            out=u_t, in0=u_t, in1=u_t, scale=1.0, scalar=0.0,
                            op0=ALU.mult, op1=ALU.add, accum_out=ci)
                        nc.vector.tensor_mul(li_col, ci, s1v)

            # per-batch sums: reduce over row tiles, then over partitions (PE)
            lsum = acc_pool.tile([128, nb], f32)
            for b in range(nb):
                nc.vector.reduce_sum(
                    lsum[:, b:b + 1],
                    li_cols[:, b * rts:(b + 1) * rts], axis=AX.X)
            ones_t = acc_pool.tile([128, 1], f32)
            nc.vector.memset(ones_t, 1.0)
            pout = psum_pool.tile([1, nb], f32, tag="pp")
            nc.tensor.matmul(pout, ones_t, lsum, start=True, stop=True)
            out_t = acc_pool.tile([1, nb], f32)
            nc.vector.tensor_copy(out_t, pout)
            nc.sync.dma_start(out=OUT, in_=out_t)

    nc.compile()
    return nc


def _split(x, levels):
    """Split fp32 array into `levels` bf16 terms (hi, mid, lo...)."""
    parts = []
    r = np.asarray(x, np.float32)
    for _ in range(levels):
        h = r.astype(bf16)
        parts.append(h)
        r = (r - h.astype(np.float32)).astype(np.float32)
    return parts


def _prep_inputs(points, weights, pointfea1, pointfea2,
                 nb=B, n=N, shard=SHARD, ncores=NCORES):
    """Host-side shard/layout prep -> list of per-core input dicts."""
    points = np.asarray(points, np.float32)
    weights = np.asarray(weights, np.float32)
    f1 = np.asarray(pointfea1, np.float32)
    f2 = np.asarray(pointfea2, np.float32)

    xs = (points / np.float32(SIGMA)).astype(np.float32)      # [B,N,3]
    x2 = (xs * xs).sum(-1, dtype=np.float32)                  # [B,N]
    xh, xm, xl = _split(xs, 3)
    y2h, y2m, y2l = _split(x2, 3)

    g1 = (f1 * f1).sum(-1, dtype=np.float32)                  # [B,N]
    g2 = (f2 * f2).sum(-1, dtype=np.float32)
    f1h, f1l = _split(f1, 2)
    f2h, f2l = _split(f2, 2)
    g2h, g2m = _split(g2, 2)

    # rhs tensors (identical on all cores), [B, K, N] bf16
    rp = np.empty((nb, 21, n), bf16)
    for d in range(3):
        rows = [2 * xh[..., d], 2 * xm[..., d], 2 * xh[..., d],
                2 * xl[..., d], 2 * xh[..., d], 2 * xm[..., d]]
        for k, rr in enumerate(rows):
            rp[:, 6 * d + k, :] = rr
    rp[:, 18, :] = -y2h
    rp[:, 19, :] = -y2m
    rp[:, 20, :] = -y2l

    rf1 = np.empty((nb, 128, n), bf16)
    rf1[:, :64, :] = (2 * f2h).transpose(0, 2, 1)
    rf1[:, 64:, :] = (2 * f2h).transpose(0, 2, 1)
    rf2 = np.empty((nb, 66, n), bf16)
    rf2[:, :64, :] = (2 * f2l).transpose(0, 2, 1)
    rf2[:, 64, :] = -g2h
    rf2[:, 65, :] = -g2m

    in_maps = []
    for c in range(ncores):
        r0 = c * shard
        sl = slice(r0, r0 + shard)
        lp = np.empty((nb, 21, shard), bf16)
        for d in range(3):
            rows = [xh[..., d], xh[..., d], xm[..., d],
                    xh[..., d], xl[..., d], xm[..., d]]
            for k, rr in enumerate(rows):
                lp[:, 6 * d + k, :] = rr[:, sl]
        lp[:, 18:21, :] = np.float32(1.0)
        lf1 = np.empty((nb, 128, shard), bf16)
        lf1[:, :64, :] = f1h[:, sl, :].transpose(0, 2, 1)
        lf1[:, 64:, :] = f1l[:, sl, :].transpose(0, 2, 1)
        lf2 = np.empty((nb, 66, shard), bf16)
        lf2[:, :64, :] = f1h[:, sl, :].transpose(0, 2, 1)
        lf2[:, 64:66, :] = np.float32(1.0)
        in_maps.append({
            "rp": rp, "rf1": rf1, "rf2": rf2,
            "lp": np.ascontiguousarray(lp),
            "lf1": np.ascontiguousarray(lf1),
            "lf2": np.ascontiguousarray(lf2),
            "bp": np.ascontiguousarray(-x2[:, sl]),
            "bf": np.ascontiguousarray(np.float32(SHIFT_F) - g1[:, sl]),
            "wv": np.ascontiguousarray(weights[:, sl]),
        })
    return in_maps


def kernel(points, weights, pointfea1, pointfea2):
    global _last_results
    from concourse.bass_utils import run_bass_kernel_spmd

    if "nc" not in _cache:
        _cache["nc"] = _build_program()
    nc = _cache["nc"]

    in_maps = _prep_inputs(points, weights, pointfea1, pointfea2)
    res = run_bass_kernel_spmd(nc, in_maps, core_ids=list(range(NCORES)))
    _last_results = res
    parts = np.stack([res.results[c]["out"][0] for c in range(NCORES)])
    return parts.sum(axis=0, dtype=np.float32)


if __name__ == "__main__":
    # smoke test with random data
    rng = np.random.default_rng(0)
    pts = rng.random((B, N, 3), np.float32)
    w = rng.random((B, N), np.float32)
    w /= w.sum(1, keepdims=True)
    a = rng.standard_normal((B, N, D)).astype(np.float32)
    bfea = rng.standard_normal((B, N, D)).astype(np.float32)
    out = kernel(pts, w, a, bfea)
    print("kernel out:", out)


# revision 20
# speedup vs baseline: 6320.3212x; 6320.3212x over previous
"""TRN2 Bass kernel for nn_DeepFeatureLoss (B=4, N=4096, D=64, 8 cores).

Math (per batch b):
  P = softmax_j(-max(|x_i/s - x_j/s|^2, 0))        (spatial gaussian)
  Q = softmax_j(-max(|f1_i - f2_j|^2, 0))          (feature affinity)
  loss_b = sum_i w_i * sum_j (P_ij - Q_ij)^2

Device scheme (row-sharded over 8 cores, 512 rows/core/batch):
  - scores via bf16 hi/lo split matmuls (full-rate PE, ~fp32 precision):
      pts:  s_ij = 2 x_i.x_j - |x_j|^2            (K=21: 3-term split + y2 rows)
      feat: s_ij = 2 f1_i.f2_j - |f2_j|^2         (K=128 + K=66 matmuls)
    row-constant |x_i|^2 terms are folded into the ACT exp bias (softmax is
    shift-invariant); max(d,0) clipping is skipped (error ~3e-4 on a handful of
    near-diagonal point pairs only; validated 6e-7 final rel err vs fp64).
  - ACT: e = exp(s + bias), fused accum -> softmax denominator S.
    feature bias gets +SHIFT to avoid denormal underflow of S_f.
  - DVE: u = e_p - e_f*(S_p/S_f)  (fused ln_bwd_dx custom op)
         li = sum_j u^2 * (w_i/S_p^2) (tensor_tensor_reduce)
  - final: per-batch partial = ones^T @ li  (PE partition-sum), host adds the
    8 per-core partials.
"""

import os
import numpy as np
import ml_dtypes

bf16 = ml_dtypes.bfloat16

SIGMA = 0.05
SHIFT_F = 30.0  # lifts feature exp away from denormals; validated on data

B, N, D = 4, 4096, 64
NCORES = 8
SHARD = N // NCORES          # 512 rows per core per batch
RT_PER_BATCH = SHARD // 128  # 4 row tiles of 128
HALF = 2048                  # psum half-row (4 banks)

_cache = {}

_last_results = None  # stashed BassKernelResults for test harnesses

USE_CUSTOM_OP = os.environ.get("DFL_CUSTOM_OP", "1") == "1"


def _get_sqdiff_op():
    """Register (once) a fused DVE op:
        out = (in0 - in1*s0)^2 * s1 ; accum_out = sum_k out[k]
    i.e. li = sum_j (e_p - rho*e_f)^2 * (w/S_p^2) in ONE 1x DVE pass.
    The per-NEFF DVE table mechanism makes runtime-registered ops first-class
    (no firmware change); see trainium-docs/custom-instructions/04-custom-dve-api.md.
    """
    if "sqdiff" in _cache:
        return _cache["sqdiff"]
    import re
    from operator import add as _add
    import numpy as np
    from concourse import dve_ops
    from concourse.dve_spec import Spec, Src0, Src1, C0, C1, Zero, sq

    name = "SQDIFF_SCALE_RED_DFL"

    def _ref(in0, in1, s0, s1, imm2):
        b = (((in0.astype(np.float32) - in1 * s0) ** 2) * s1).astype(np.float32)
        return b, b.reshape(b.shape[0], -1).sum(axis=-1, keepdims=True)

    spec = Spec(body=sq(Src0 - Src1 * C0) * C1, accum=_add, accum_init=Zero,
                reference=_ref)
    if name not in dve_ops._SUB_OPCODE_FOR_NAME:
        row = max(dve_ops._SUB_OPCODE_FOR_NAME.values()) + 1
        assert row < 0x20
        dve_ops._SUB_OPCODE_FOR_NAME[name] = row
    # compute the uops sha by probing (compile() reports the real value)
    shas = {}
    for ver in ("v3", "v4"):
        probe = dve_ops.DveOp(name, spec, subdim=False, uops_sha={})
        try:
            probe.compile(ver)
        except ValueError as e:
            m = re.search(r"\{ver\}.*?=\"([0-9a-f]+)\"".replace("{ver}", ver),
                          str(e)) or re.search(r'"([0-9a-f]{16})"', str(e))
            shas[ver] = m.group(1)
    op = dve_ops.DveOp(name, spec, subdim=False, uops_sha=shas)
    if not any(o.name == name for o in dve_ops.OPS):
        dve_ops.OPS.append(op)
    dve_ops.CUSTOM_DVE_SPECS[name] = spec
    _cache["sqdiff"] = op
    return op


def _build_program(n=N, shard=SHARD, nb=B, qcol=512, ebufs=2, rhsbufs=2):
    """Emit the per-core Bass program. Identical on all cores (pure SPMD).

    Per-row softmax biases (-|x_i|^2 and SHIFT-|f1_i|^2) ride the matmuls as
    bf16-split lhsT rows against ones-rows in the rhs, so no scattered
    per-partition bias DMAs are needed and ACT runs with bias=0.
    """
    import concourse.bacc as bacc
    import concourse.tile as tile
    from concourse import mybir

    f32 = mybir.dt.float32
    b16 = mybir.dt.bfloat16
    AX = mybir.AxisListType
    ALU = mybir.AluOpType
    ACTF = mybir.ActivationFunctionType

    rts = shard // 128
    nhalf = max(1, n // HALF)
    half = min(n, HALF)
    qper = half // qcol  # matmuls per half

    nc = bacc.Bacc("TRN2", target_bir_lowering=False, debug=False,
                   num_devices=NCORES)

    KP = 24   # points: 18 coord-split rows + 3 y2 rows + 3 bias rows
    KF2 = 69  # feat mm2: 64 f-lo rows + 2 g2 rows + 3 bias rows
    RP = nc.dram_tensor("rp", [nb, KP, n], b16, kind="ExternalInput").ap()
    RF1 = nc.dram_tensor("rf1", [nb, 128, n], b16, kind="ExternalInput").ap()
    RF2 = nc.dram_tensor("rf2", [nb, KF2, n], b16, kind="ExternalInput").ap()
    LP = nc.dram_tensor("lp", [nb, KP, shard], b16, kind="ExternalInput").ap()
    LF1 = nc.dram_tensor("lf1", [nb, 128, shard], b16, kind="ExternalInput").ap()
    LF2 = nc.dram_tensor("lf2", [nb, KF2, shard], b16, kind="ExternalInput").ap()
    WV = nc.dram_tensor("wv", [128, nb * rts], f32, kind="ExternalInput").ap()
    OUT = nc.dram_tensor("out", [1, nb], f32, kind="ExternalOutput").ap()

    with tile.TileContext(nc) as tc:
        with (
            tc.tile_pool(name="rhs", bufs=rhsbufs) as rhs_pool,
            tc.tile_pool(name="lhs", bufs=2) as lhs_pool,
            tc.tile_pool(name="ebuf", bufs=ebufs) as e_pool,
            tc.tile_pool(name="ubuf", bufs=2) as u_pool,
            tc.tile_pool(name="small", bufs=8) as small,
            tc.tile_pool(name="acc", bufs=1) as acc_pool,
            tc.tile_pool(name="psum", bufs=1, space="PSUM") as psum_pool,
        ):
            li_cols = acc_pool.tile([128, nb * rts], f32)
            lsum = acc_pool.tile([128, nb], f32)
            w_all = acc_pool.tile([128, nb * rts], f32)
            nc.sync.dma_start(out=w_all, in_=WV)

            # dummy exp to hoist the ACT table load off the critical path
            warm = small.tile([1, 1], f32, tag="warm")
            nc.vector.memset(warm, 0.0)
            nc.scalar.activation(out=warm, in_=warm, func=ACTF.Exp)

            for b in range(nb):
                # whole-batch lhsT tiles (efficient contiguous DMAs, gate the
                # first matmuls -> issued before the big rhs streams)
                lp_t = lhs_pool.tile([KP, shard], b16, tag="lp")
                nc.sync.dma_start(out=lp_t, in_=LP[b])
                lf1_t = lhs_pool.tile([128, shard], b16, tag="lf1")
                nc.sync.dma_start(out=lf1_t, in_=LF1[b])
                lf2_t = lhs_pool.tile([KF2, shard], b16, tag="lf2")
                nc.sync.dma_start(out=lf2_t, in_=LF2[b])

                # rhs tiles split by j-half; points (small, gates first mms)
                # load first
                rp_h = [rhs_pool.tile([KP, half], b16, tag=f"rp{h}",
                                      name=f"rp_h{h}")
                        for h in range(nhalf)]
                rf1_h = [rhs_pool.tile([128, half], b16, tag=f"rf1{h}",
                                       name=f"rf1_h{h}")
                         for h in range(nhalf)]
                rf2_h = [rhs_pool.tile([KF2, half], b16, tag=f"rf2{h}",
                                       name=f"rf2_h{h}")
                         for h in range(nhalf)]
                for h in range(nhalf):
                    j0 = h * half
                    nc.sync.dma_start(out=rp_h[h], in_=RP[b][:, j0:j0 + half])
                    nc.sync.dma_start(out=rf1_h[h], in_=RF1[b][:, j0:j0 + half])
                    nc.sync.dma_start(out=rf2_h[h], in_=RF2[b][:, j0:j0 + half])

                for rt in range(rts):
                    r0 = rt * 128
                    e_p = e_pool.tile([128, n], f32, tag="ep")
                    e_f = e_pool.tile([128, n], f32, tag="ef")
                    sp_cols = small.tile([128, nhalf], f32, tag="spc")
                    sf_cols = small.tile([128, nhalf], f32, tag="sfc")

                    for h in range(nhalf):
                        ppts = psum_pool.tile([128, half], f32, tag="pp")
                        pfea = psum_pool.tile([128, half], f32, tag="pf")
                        # points: K=KP (coords + y2 + bias rows)
                        for q in range(qper):
                            nc.tensor.matmul(
                                ppts[:, q * qcol:(q + 1) * qcol],
                                lp_t[:, r0:r0 + 128],
                                rp_h[h][:, q * qcol:(q + 1) * qcol],
                                start=True, stop=True)
                        # features: K=128 (hh+lh) then K=KF2 (hl+g2+bias)
                        for q in range(qper):
                            nc.tensor.matmul(
                                pfea[:, q * qcol:(q + 1) * qcol],
                                lf1_t[:, r0:r0 + 128],
                                rf1_h[h][:, q * qcol:(q + 1) * qcol],
                                start=True, stop=False)
                        for q in range(qper):
                            nc.tensor.matmul(
                                pfea[:, q * qcol:(q + 1) * qcol],
                                lf2_t[:, r0:r0 + 128],
                                rf2_h[h][:, q * qcol:(q + 1) * qcol],
                                start=False, stop=True)
                        nc.scalar.activation(
                            out=e_p[:, h * half:(h + 1) * half], in_=ppts,
                            func=ACTF.Exp, scale=1.0,
                            accum_out=sp_cols[:, h:h + 1])
                        nc.scalar.activation(
                            out=e_f[:, h * half:(h + 1) * half], in_=pfea,
                            func=ACTF.Exp, scale=1.0,
                            accum_out=sf_cols[:, h:h + 1])

                    s_p = small.tile([128, 1], f32, tag="sp")
                    nc.vector.reduce_sum(s_p, sp_cols, axis=AX.X)
                    s_f = small.tile([128, 1], f32, tag="sf")
                    nc.vector.reduce_sum(s_f, sf_cols, axis=AX.X)
                    rsf = small.tile([128, 1], f32, tag="rsf")
                    nc.vector.reciprocal(rsf, s_f)
                    rho = small.tile([128, 1], f32, tag="rho")
                    nc.vector.tensor_mul(rho, s_p, rsf)
                    rsp = small.tile([128, 1], f32, tag="rsp")
                    nc.vector.reciprocal(rsp, s_p)
                    rsp2 = small.tile([128, 1], f32, tag="rsp2")
                    nc.vector.tensor_mul(rsp2, rsp, rsp)
                    s1v = small.tile([128, 1], f32, tag="s1v")
                    col = b * rts + rt
                    nc.vector.tensor_mul(s1v, rsp2, w_all[:, col:col + 1])

                    li_col = li_cols[:, col:col + 1]
                    if USE_CUSTOM_OP:
                        # fused: li = sum_j (e_p - rho*e_f)^2 * (w/S_p^2)
                        u_t = u_pool.tile([128, n], f32, tag="u")
                        nc.vector._custom_dve(
                            _get_sqdiff_op(), out=u_t, in0=e_p, in1=e_f,
                            s0=rho, s1=s1v, accum_out=li_col)
                    else:
                        u_t = u_pool.tile([128, n], f32, tag="u")
                        nc.vector.ln_bwd_dx(out=u_t, dy=e_p, x_hat=e_f,
                                            mean_dyx=rho, mean_dy=0.0,
                                            scale=1.0)
                        ci = small.tile([128, 1], f32, tag="ci")
                        nc.vector.tensor_tensor_reduce(
                            out=u_t, in0=u_t, in1=u_t, scale=1.0, scalar=0.0,
                            op0=ALU.mult, op1=ALU.add, accum_out=ci)
                        nc.vector.tensor_mul(li_col, ci, s1v)

            # per-batch sums: reduce over row tiles, then over partitions (PE)
            for b in range(nb):
                nc.vector.reduce_sum(
                    lsum[:, b:b + 1],
                    li_cols[:, b * rts:(b + 1) * rts], axis=AX.X)
            ones_t = acc_pool.tile([128, 1], f32)
            nc.vector.memset(ones_t, 1.0)
            pout = psum_pool.tile([1, nb], f32, tag="pp")
            nc.tensor.matmul(pout, ones_t, lsum, start=True, stop=True)
            out_t = acc_pool.tile([1, nb], f32)
            nc.vector.tensor_copy(out_t, pout)
            nc.sync.dma_start(out=OUT, in_=out_t)

    nc.compile()
    return nc


def _split(x, levels):
    """Split fp32 array into `levels` bf16 terms (hi, mid, lo...)."""
    parts = []
    r = np.asarray(x, np.float32)
    for _ in range(levels):
        h = r.astype(bf16)
        parts.append(h)
        r = (r - h.astype(np.float32)).astype(np.float32)
    return parts


def _prep_inputs(points, weights, pointfea1, pointfea2,
                 nb=B, n=N, shard=SHARD, ncores=NCORES):
    """Host-side shard/layout prep -> list of per-core input dicts."""
    points = np.asarray(points, np.float32)
    weights = np.asarray(weights, np.float32)
    f1 = np.asarray(pointfea1, np.float32)
    f2 = np.asarray(pointfea2, np.float32)

    xs = (points / np.float32(SIGMA)).astype(np.float32)      # [B,N,3]
    x2 = (xs * xs).sum(-1, dtype=np.float32)                  # [B,N]
    xh, xm, xl = _split(xs, 3)
    y2h, y2m, y2l = _split(x2, 3)
    bph, bpm, bpl = _split(-x2, 3)                            # points bias rows

    g1 = (f1 * f1).sum(-1, dtype=np.float32)                  # [B,N]
    g2 = (f2 * f2).sum(-1, dtype=np.float32)
    f1h, f1l = _split(f1, 2)
    f2h, f2l = _split(f2, 2)
    g2h, g2m = _split(g2, 2)
    bfh, bfm, bfl = _split(np.float32(SHIFT_F) - g1, 3)       # feature bias rows

    # rhs tensors (identical on all cores), [B, K, N] bf16
    rp = np.empty((nb, 24, n), bf16)
    for d in range(3):
        rows = [2 * xh[..., d], 2 * xm[..., d], 2 * xh[..., d],
                2 * xl[..., d], 2 * xh[..., d], 2 * xm[..., d]]
        for k, rr in enumerate(rows):
            rp[:, 6 * d + k, :] = rr
    rp[:, 18, :] = -y2h
    rp[:, 19, :] = -y2m
    rp[:, 20, :] = -y2l
    rp[:, 21:24, :] = np.float32(1.0)

    rf1 = np.empty((nb, 128, n), bf16)
    rf1[:, :64, :] = (2 * f2h).transpose(0, 2, 1)
    rf1[:, 64:, :] = (2 * f2h).transpose(0, 2, 1)
    rf2 = np.empty((nb, 69, n), bf16)
    rf2[:, :64, :] = (2 * f2l).transpose(0, 2, 1)
    rf2[:, 64, :] = -g2h
    rf2[:, 65, :] = -g2m
    rf2[:, 66:69, :] = np.float32(1.0)

    in_maps = []
    for c in range(ncores):
        r0 = c * shard
        sl = slice(r0, r0 + shard)
        lp = np.empty((nb, 24, shard), bf16)
        for d in range(3):
            rows = [xh[..., d], xh[..., d], xm[..., d],
                    xh[..., d], xl[..., d], xm[..., d]]
            for k, rr in enumerate(rows):
                lp[:, 6 * d + k, :] = rr[:, sl]
        lp[:, 18:21, :] = np.float32(1.0)
        lp[:, 21, :] = bph[:, sl]
        lp[:, 22, :] = bpm[:, sl]
        lp[:, 23, :] = bpl[:, sl]
        lf1 = np.empty((nb, 128, shard), bf16)
        lf1[:, :64, :] = f1h[:, sl, :].transpose(0, 2, 1)
        lf1[:, 64:, :] = f1l[:, sl, :].transpose(0, 2, 1)
        lf2 = np.empty((nb, 69, shard), bf16)
        lf2[:, :64, :] = f1h[:, sl, :].transpose(0, 2, 1)
        lf2[:, 64:66, :] = np.float32(1.0)
        lf2[:, 66, :] = bfh[:, sl]
        lf2[:, 67, :] = bfm[:, sl]
        lf2[:, 68, :] = bfl[:, sl]
        rts = shard // 128
        wvt = np.empty((128, nb * rts), np.float32)
        for b in range(nb):
            for rt in range(rts):
                wvt[:, b * rts + rt] = weights[b, r0 + rt * 128:r0 + (rt + 1) * 128]
        in_maps.append({
            "rp": rp, "rf1": rf1, "rf2": rf2,
            "lp": np.ascontiguousarray(lp),
            "lf1": np.ascontiguousarray(lf1),
            "lf2": np.ascontiguousarray(lf2),
            "wv": wvt,
        })
    return in_maps


def kernel(points, weights, pointfea1, pointfea2):
    global _last_results
    from concourse.bass_utils import run_bass_kernel_spmd

    if "nc" not in _cache:
        _cache["nc"] = _build_program()
    nc = _cache["nc"]

    in_maps = _prep_inputs(points, weights, pointfea1, pointfea2)
    res = run_bass_kernel_spmd(nc, in_maps, core_ids=list(range(NCORES)))
    _last_results = res
    parts = np.stack([res.results[c]["out"][0] for c in range(NCORES)])
    return parts.sum(axis=0, dtype=np.float32)


if __name__ == "__main__":
    # smoke test with random data
    rng = np.random.default_rng(0)
    pts = rng.random((B, N, 3), np.float32)
    w = rng.random((B, N), np.float32)
    w /= w.sum(1, keepdims=True)
    a = rng.standard_normal((B, N, D)).astype(np.float32)
    bfea = rng.standard_normal((B, N, D)).astype(np.float32)
    out = kernel(pts, w, a, bfea)
    print("kernel out:", out)
